# revision 42
# baseline (speedup 1.0000x reference)
"""Causal self-attention Trainium2 kernel (v2).

Shards batch(4) x head-group(2) across 8 NeuronCores. Each core computes, for
its batch b and its 8 heads:
    qkv = x[b] @ w_qkv_shard (+ b_qkv_shard)
    per head: S^T = k q^T / 8 (causal), P^T = exp(S^T), [o^T; den] = [v|1]^T P^T
    partial_out = o_all @ w_proj_shard
Host sums the two head-group partials per batch and adds b_proj.

v2 changes over v1:
  - bf16 matmul operands (half the DMA/SBUF traffic, lower PE power; the
    chip duty-throttles to 4/8 speed under sustained fp32r load).
  - q^T/k^T stay in SBUF (v1 spilled 8 MB to DRAM and reloaded).
  - tight causal tiling: the 4 diagonal-block score/AV matmuls per chunk
    only cover valid columns (N = 512-128j) instead of full 512.
  - per-chunk normalization: reciprocal straight from the PSUM denom row,
    partition_broadcast from partition 0, all-bf16 normalize mul.
  - qkv matmul groups for pair p+1 are interleaved between attention
    chunks of pair p so the in-order PE has work while ACT (exp) catches
    up (ACT is the per-head co-bottleneck).
"""

import numpy as np
import ml_dtypes
from contextlib import ExitStack

import concourse.bass as bass
import concourse.bacc as bacc
import concourse.mybir as mybir
import concourse.tile as tile
from concourse import bass_utils
from concourse.masks import make_upper_triangular

D = 1024
T = 2048
B = 4
NH = 16          # global heads
HD = 64
NCORES = 8
HL = 8           # heads per core (local)
DS = HL * HD     # 512: per-core head-feature width

F32 = mybir.dt.float32
BF16 = mybir.dt.bfloat16
DT = BF16
EXPF = mybir.ActivationFunctionType.Exp

TQ = 512         # tq chunk width (scores free dim)
NTT = T // 128   # 16 token tiles
NK = D // 128    # 8 contraction tiles
VW = HL * (HD + 1) + 64   # v_ext width (ones col per head + slack)


def _build(with_bias: bool):
    nc = bacc.Bacc("TRN2", target_bir_lowering=False, debug=False,
                   num_devices=NCORES)
    KROWS = D + 1 if with_bias else D
    xT = nc.dram_tensor("xT", [KROWS, T], DT, kind="ExternalInput")
    w = nc.dram_tensor("w", [KROWS, 3 * DS], DT, kind="ExternalInput")
    wp = nc.dram_tensor("wp", [DS, D], DT, kind="ExternalInput")
    out = nc.dram_tensor("out", [T, D], F32, kind="ExternalOutput")

    with tile.TileContext(nc) as tc, ExitStack() as ctx:
        # persistent pools
        xpool = ctx.enter_context(tc.tile_pool(name="xp", bufs=1))
        wpool = ctx.enter_context(tc.tile_pool(name="wpa", bufs=1))
        qkpool = ctx.enter_context(tc.tile_pool(name="qkp", bufs=1))
        vext_pool = ctx.enter_context(tc.tile_pool(name="vext", bufs=1))
        ot_pool = ctx.enter_context(tc.tile_pool(name="ot", bufs=1))
        wpp = ctx.enter_context(tc.tile_pool(name="wpp", bufs=1))
        misc = ctx.enter_context(tc.tile_pool(name="misc", bufs=1))
        pt_pool = ctx.enter_context(tc.tile_pool(name="pt", bufs=3))
        osb = ctx.enter_context(tc.tile_pool(name="osb", bufs=9))
        nrm = ctx.enter_context(tc.tile_pool(name="nrm", bufs=2))
        cstg = ctx.enter_context(tc.tile_pool(name="cstg", bufs=4))
        # PSUM: aps(2) + scp(2x2) + osp(2) = 8 banks
        aps = ctx.enter_context(tc.tile_pool(name="aps", bufs=2,
                                             space="PSUM"))
        scp = ctx.enter_context(tc.tile_pool(name="scp", bufs=2,
                                             space="PSUM"))
        osp = ctx.enter_context(tc.tile_pool(name="osp", bufs=2,
                                             space="PSUM"))

        mask = misc.tile([128, 128], DT, tag="mask", name="mask")
        make_upper_triangular(nc, mask[:], val=1.0, diag=True)
        # preload the gpsimd broadcast ucode lib during the DMA-wait
        # startup window: switching gpsimd op families mid-kernel costs
        # a ~6.5us UNLOAD_LIB/LOAD_LIB stall, so gpsimd runs ONLY
        # partition_broadcast afterwards (all memsets live on DVE)
        bcs_warm = misc.tile([64, 128], DT, tag="bcs_warm",
                             name="bcs_warm")
        nc.gpsimd.partition_broadcast(bcs_warm[:], mask[0:1, :])

        # ---------------- input DMA (split, need-ordered) ----------------
        # wtv + x chunk 0 land first so the v-projection starts within a
        # few us; wtqk/wpt stream in behind.
        wtv = []
        for k in range(NK):
            t_ = wpool.tile([128, 512], DT, tag=f"wtv{k}", name=f"wtv{k}")
            nc.sync.dma_start(t_[:],
                              w.ap()[k * 128:(k + 1) * 128, 2 * DS:3 * DS])
            wtv.append(t_)
        xtc = [[] for _ in range(NK)]
        for ch in range(4):
            for k in range(NK):
                t_ = xpool.tile([128, 512], DT, tag=f"xt{k}_{ch}",
                                name=f"xt{k}_{ch}")
                nc.sync.dma_start(
                    t_[:], xT.ap()[k * 128:(k + 1) * 128,
                                   ch * 512:(ch + 1) * 512])
                xtc[k].append(t_)
        wtqk = []
        for k in range(NK):
            t_ = wpool.tile([128, 2 * DS], DT, tag=f"wtqk{k}",
                            name=f"wtqk{k}")
            nc.sync.dma_start(t_[:], w.ap()[k * 128:(k + 1) * 128,
                                            0:2 * DS])
            wtqk.append(t_)
        if with_bias:
            xb = xpool.tile([1, T], DT, tag="xb", name="xb")
            nc.sync.dma_start(xb[:], xT.ap()[D:D + 1, :])
            wbv = wpool.tile([1, 512], DT, tag="wbv", name="wbv")
            nc.sync.dma_start(wbv[:], w.ap()[D:D + 1, 2 * DS:3 * DS])
            wbqk = wpool.tile([1, 2 * DS], DT, tag="wbqk", name="wbqk")
            nc.sync.dma_start(wbqk[:], w.ap()[D:D + 1, 0:2 * DS])
        wpt = []
        for k in range(DS // 128):
            t_ = wpp.tile([128, D], DT, tag=f"wpt{k}", name=f"wpt{k}")
            nc.sync.dma_start(t_[:], wp.ap()[k * 128:(k + 1) * 128, :])
            wpt.append(t_)

        v_ext = [vext_pool.tile([128, VW], DT, tag=f"vext{t}",
                                name=f"vext{t}")
                 for t in range(NTT)]
        qk = [qkpool.tile([128, T], DT, tag=f"qk{m}", name=f"qk{m}")
              for m in range(2 * DS // 128)]
        oT = [ot_pool.tile([128, T], DT, tag=f"ot{m}", name=f"ot{m}")
              for m in range(4)]

        # ---------------- phase A: v projection ----------------
        # v : [T, DS] token-major, packed as [128, 8*(64+1)] with ones col
        for t in range(NTT):
            ps = aps.tile([128, 512], F32, tag="aps", name="aps")
            for k in range(NK):
                nc.tensor.matmul(
                    ps[:],
                    lhsT=xtc[k][t // 4][:, (t % 4) * 128:
                                        (t % 4 + 1) * 128],
                    rhs=wtv[k][:],
                    start=(k == 0),
                    stop=(k == NK - 1 and not with_bias))
            if with_bias:
                nc.tensor.matmul(
                    ps[:],
                    lhsT=xb[0:1, t * 128:(t + 1) * 128],
                    rhs=wbv[0:1, :],
                    start=False, stop=True)
            # only the ones-columns and the hl=7 slice overhang need
            # initialization, not the whole tile
            ones_cols = v_ext[t][:, 0:HL * (HD + 1)].rearrange(
                "p (h c) -> p h c", h=HL)[:, :, HD:HD + 1]
            nc.vector.memset(ones_cols, 1.0)
            nc.vector.memset(v_ext[t][:, HL * (HD + 1):], 1.0)
            dst = v_ext[t][:, 0:HL * (HD + 1)].rearrange(
                "p (h c) -> p h c", h=HL)[:, :, 0:HD]
            src = ps.rearrange("p (h c) -> p h c", h=HL)
            nc.scalar.copy(dst, src)

        # ---------------- phase A': q^T/k^T projection ----------------
        # qk[m] rows: m in 0..3 = q^T head pairs, 4..7 = k^T head pairs.
        # emitted lazily: pair 0 upfront, pair p+1 interleaved into
        # attention of pair p (one n-group after each attention chunk).
        def emit_qk_group(m, n):
            ps = aps.tile([128, 512], F32, tag="aps", name="aps")
            for k in range(NK):
                nc.tensor.matmul(
                    ps[:],
                    lhsT=wtqk[k][:, m * 128:(m + 1) * 128],
                    rhs=xtc[k][n][:],
                    start=(k == 0),
                    stop=(k == NK - 1 and not with_bias))
            if with_bias:
                nc.tensor.matmul(
                    ps[:],
                    lhsT=wbqk[0:1, m * 128:(m + 1) * 128],
                    rhs=xb[0:1, n * 512:(n + 1) * 512],
                    start=False, stop=True)
            nc.vector.tensor_copy(qk[m][:, n * 512:(n + 1) * 512], ps[:])

        def qk_groups_for_pair(p):
            for m in (p, 4 + p):
                for n in range(T // 512):
                    yield (m, n)

        for m, n in qk_groups_for_pair(0):
            emit_qk_group(m, n)

        # ---------------- phase C groups (emitted inside pair 3) -------
        def emit_c_group(t, n, use_act=False):
            ps = aps.tile([128, 512], F32, tag="aps", name="aps")
            for k in range(DS // 128):
                nc.tensor.matmul(
                    ps[:],
                    lhsT=oT[k][:, t * 128:(t + 1) * 128],
                    rhs=wpt[k][:, n * 512:(n + 1) * 512],
                    start=(k == 0), stop=(k == DS // 128 - 1))
            ot_s = cstg.tile([128, 512], F32, tag="cstg", name="cstg")
            if use_act:
                nc.scalar.copy(ot_s[:], ps[:])
            else:
                nc.vector.tensor_copy(ot_s[:], ps[:])
            nc.sync.dma_start(
                out.ap()[t * 128:(t + 1) * 128,
                         n * 512:(n + 1) * 512], ot_s[:])

        # ---------------- phase B: attention ----------------
        # Head i's normalization (recip + 4x broadcast/mul) is deferred
        # and drained into head i+1's chunk slots so the in-order DVE /
        # gpsimd queues never burst at a head boundary. The last head's
        # norm interleaves with phase C in the tail.
        norm_steps = []

        def head_norm_steps(p, po, dnl, osbsl):
            rcall = nrm.tile([128, 512], F32, tag="rcall", name="rcall")

            def do_recip():
                nc.vector.reciprocal(rcall[:], dnl[:])

            steps = [do_recip]
            for cc in range(T // TQ):
                def do_norm(cc=cc):
                    rcb = nrm.tile([1, 512], DT, tag="rcb", name="rcb")
                    nc.vector.tensor_copy(
                        rcb[:], rcall[32 * cc:32 * cc + 1, :])
                    bcs = nrm.tile([64, 512], DT, tag="bcs", name="bcs")
                    nc.gpsimd.partition_broadcast(bcs[:], rcb[:])
                    dst = oT[p][po:po + 64, cc * TQ:(cc + 1) * TQ]
                    nc.vector.tensor_mul(dst, osbsl[cc][:], bcs[:])

                steps.append(do_norm)
            return steps

        for p in range(4):  # head pairs
            qt, kt = qk[p], qk[4 + p]
            pending_qk = list(qk_groups_for_pair(p + 1)) if p < 3 else []

            for h01 in range(2):
                hl = 2 * p + h01          # local head
                po = h01 * 64             # partition offset in qt/kt
                last = (p == 3 and h01 == 1)

                # chunk-c denominator parked at partition 32c (engine APs
                # may only start at partitions 0/32/64/96); one batched
                # reciprocal per head covers all 4 chunks. The last head
                # instead normalizes per chunk (reciprocal straight from
                # PSUM) so phase C can start right behind each chunk.
                if not last:
                    dn = nrm.tile([128, 512], F32, tag="dn", name="dn")
                    nc.vector.memset(dn[:], 1.0)
                osbs = []

                for c in range(T // TQ):  # tq chunks
                    ntk = 4 * c + 4       # token tiles for this chunk
                    op = osp.tile([128, 512], F32, tag="osp", name="osp")

                    # build (t, q0) list; group into pairs sharing one
                    # [128,1024] PSUM tile / one-two ACT calls
                    tiles = []
                    for t in range(ntk):
                        q0 = max(0, 128 * (t - 4 * c))
                        tiles.append((t, q0))
                    groups = [tiles[i:i + 2] for i in range(0, ntk, 2)]

                    # AV accumulation order: the PSUM bank's first and
                    # last writes must be full-width (start/stop) for the
                    # sim's group model, so t=0 goes last for c>0. For
                    # c=0 every tile is diagonal: memset the pt groups
                    # and accumulate full-width instead.
                    av_full = (c == 0)
                    if c > 0:
                        av_order = tiles[1:] + tiles[:1]
                    else:
                        av_order = list(tiles)
                    av_first = av_order[0][0]
                    av_last = av_order[-1][0]
                    pt_of = {}            # t -> pt AP for AV
                    av_idx = 0

                    def emit_av_one(t, q0):
                        a0 = 0 if av_full else q0
                        nc.tensor.matmul(
                            op[:, a0:512],
                            lhsT=v_ext[t][:, hl * (HD + 1):
                                          hl * (HD + 1) + 128],
                            rhs=pt_of[t],
                            start=(t == av_first),
                            stop=(t == av_last),
                            skip_group_check=(a0 > 0))

                    for gi, g in enumerate(groups):
                        sc = scp.tile([128, 1024], F32, tag="scp",
                                      name="scp")
                        # group 0's pt must survive until the end of the
                        # chunk (its AV for t=0 is emitted last)
                        pt = pt_pool.tile(
                            [128, 1024], DT,
                            tag=("pt0" if gi == 0 else "pt"),
                            name="pt")
                        if av_full:
                            nc.vector.memset(pt[:], 0.0)
                        for s, (t, q0) in enumerate(g):
                            off = s * 512
                            nc.tensor.matmul(
                                sc[:, off + q0:off + 512],
                                lhsT=kt[po:po + 64,
                                        t * 128:(t + 1) * 128],
                                rhs=qt[po:po + 64,
                                       c * TQ + q0:(c + 1) * TQ],
                                start=True, stop=True)
                        # exp (scale folded in); full-width pairs get one
                        # ACT call, diagonal tiles get their own
                        if all(q0 == 0 for (_, q0) in g):
                            wdt = len(g) * 512
                            nc.scalar.activation(pt[:, :wdt], sc[:, :wdt],
                                                 EXPF, scale=0.125)
                        else:
                            for s, (t, q0) in enumerate(g):
                                off = s * 512
                                nc.scalar.activation(
                                    pt[:, off + q0:off + 512],
                                    sc[:, off + q0:off + 512],
                                    EXPF, scale=0.125)
                        # mask diagonal squares
                        for s, (t, q0) in enumerate(g):
                            if t - 4 * c >= 0:
                                off = s * 512
                                blk = pt[:, off + q0:off + q0 + 128]
                                nc.vector.tensor_mul(blk, blk, mask[:])
                        for s, (t, q0) in enumerate(g):
                            a0 = 0 if av_full else q0
                            pt_of[t] = pt[:, s * 512 + a0:s * 512 + 512]
                        # emit AVs whose exp group is already done,
                        # keeping one group of pipeline lag
                        while (av_idx < len(av_order)
                               and av_order[av_idx][0] // 2 < gi):
                            emit_av_one(*av_order[av_idx])
                            av_idx += 1
                    # PE filler goes BEFORE the AV drain: the remaining
                    # AVs wait on the last exps, so give the PE real
                    # work (next pair's qkv) to chew on meanwhile
                    if pending_qk:
                        emit_qk_group(*pending_qk.pop(0))
                    while av_idx < len(av_order):
                        emit_av_one(*av_order[av_idx])
                        av_idx += 1

                    # evacuate o^T (bf16) + park the denominator row;
                    # normalization is deferred into the next head
                    o_raw = osb.tile([HD, 512], DT, tag="osb", name="osb")
                    nc.vector.tensor_copy(o_raw[:], op[0:HD, :])
                    # drain the previous head's deferred norm steps
                    # (before the last head's C groups: they read the
                    # previous head's oT columns)
                    for _ in range(2):
                        if norm_steps:
                            norm_steps.pop(0)()

                    if not last:
                        nc.vector.tensor_copy(dn[32 * c:32 * c + 1, :],
                                              op[HD:HD + 1, :])
                        osbs.append(o_raw)
                    else:
                        # inline per-chunk normalize + output projection
                        rcf = nrm.tile([1, 512], F32, tag="rcf",
                                       name="rcf")
                        nc.vector.reciprocal(rcf[:], op[HD:HD + 1, :])
                        rcb = nrm.tile([1, 512], DT, tag="rcb",
                                       name="rcb")
                        nc.vector.tensor_copy(rcb[:], rcf[:])
                        bcs = nrm.tile([64, 512], DT, tag="bcs",
                                       name="bcs")
                        nc.gpsimd.partition_broadcast(bcs[:], rcb[:])
                        dst = oT[p][po:po + 64, c * TQ:(c + 1) * TQ]
                        nc.vector.tensor_mul(dst, o_raw[:], bcs[:])
                        for tt in range(4 * c, 4 * c + 4):
                            for n in range(D // 512):
                                emit_c_group(tt, n, use_act=(c == 3))

                if not last:
                    norm_steps.extend(head_norm_steps(p, po, dn, osbs))

            while pending_qk:
                emit_qk_group(*pending_qk.pop(0))

    nc.compile()
    return nc


_CACHE = {}


def _get_nc(with_bias: bool):
    if with_bias not in _CACHE:
        _CACHE[with_bias] = _build(with_bias)
    return _CACHE[with_bias]


def make_in_maps(x, w_qkv, b_qkv, w_proj, with_bias):
    """Per-core input dicts (host-side shard + transpose + pack)."""
    x = np.asarray(x, dtype=np.float32)
    w_qkv = np.asarray(w_qkv, dtype=np.float32)
    b_qkv = np.asarray(b_qkv, dtype=np.float32)
    w_proj = np.asarray(w_proj, dtype=np.float32)
    in_maps = []
    for core in range(NCORES):
        b, hg = divmod(core, 2)
        cols = np.r_[hg * DS:hg * DS + DS,
                     D + hg * DS:D + hg * DS + DS,
                     2 * D + hg * DS:2 * D + hg * DS + DS]
        w_s = w_qkv[:, cols]                      # [D, 3*DS]
        xTc = np.ascontiguousarray(x[b].T)        # [D, T]
        if with_bias:
            xTc = np.concatenate([xTc, np.ones((1, T), np.float32)], axis=0)
            w_s = np.concatenate([w_s, b_qkv[cols][None, :]], axis=0)
        cast = lambda a: np.ascontiguousarray(a).astype(ml_dtypes.bfloat16)
        in_maps.append({
            "xT": cast(xTc),
            "w": cast(w_s),
            "wp": cast(w_proj[hg * DS:(hg + 1) * DS, :]),
        })
    return in_maps


LAST_EXEC_TIME_NS = None


def kernel(x, w_qkv, b_qkv, w_proj, b_proj):
    global LAST_EXEC_TIME_NS
    with_bias = bool(np.any(np.asarray(b_qkv)))
    nc = _get_nc(with_bias)
    in_maps = make_in_maps(x, w_qkv, b_qkv, w_proj, with_bias)
    res = bass_utils.run_bass_kernel_spmd(
        nc, in_maps, core_ids=list(range(NCORES)))
    LAST_EXEC_TIME_NS = res.exec_time_ns
    b_proj = np.asarray(b_proj, dtype=np.float32)
    out = np.empty((B, T, D), dtype=np.float32)
    for b in range(B):
        out[b] = (res.results[2 * b]["out"] + res.results[2 * b + 1]["out"]
                  + b_proj)
    return out
